# revision 48
# baseline (speedup 1.0000x reference)
"""Bahdanau additive attention on TRN2 (Bass/Tile), 8-core data-parallel.

Math (per batch row b):
    qp   = query @ Wq.T                      # [H]
    kp_s = keys[s] @ Wk.T                    # [S, H]
    e_s  = v . tanh(qp + kp_s)               # [S]
    attn = softmax(mask(e))                  # [S]  (all-pad rows -> 0)
    ctx  = attn @ keys                       # [K]

Sharding: B=64 batches split 8 ways (8 per core); weights replicated.

Per-core kernel layout strategy:
  - The projection contracts over k, so keys tiles are PE-transposed
    ([s,k] -> [k,s]) and the matmul runs with Wk^T chunks stationary,
    producing kp^T [h, s] in PSUM (fp32r matmuls: full-rate with fp32
    storage).
  - tanh(kp^T + qp^T) is fused on ScalarE via activation bias (per
    partition = per h).
  - energy = v . tanh(...) is a second PE matmul contracting h.
  - softmax per batch row runs on partition 0 (tiny: [1, S]).
  - context re-streams keys in natural [s, k] layout and contracts s
    with the attention column as the stationary operand.
"""

import os
import sys

import numpy as np

for _p in ("/opt/trn_rl_repo", os.path.expanduser("~/.axon_site/_ro/trn_rl_repo")):
    if os.path.isdir(_p) and _p not in sys.path:
        sys.path.append(_p)

from contextlib import ExitStack

import concourse.bacc as bacc
import concourse.bass as bass
import concourse.mybir as mybir
import concourse.tile as tile
from concourse import bass_isa, bass_utils
from concourse.masks import make_identity

F32 = mybir.dt.float32
F32R = mybir.dt.float32r
BF16 = mybir.dt.bfloat16
U8 = mybir.dt.uint8
P = 128

N_CORES = 8
FULL_B, FULL_S, FULL_H, FULL_K = 64, 2048, 1024, 1024

NEG_BIG = -30.0  # masked-energy fill; |energy| << 30 so never collides


def _emit(ctx, tc, io, B, S, K, H, SB):
    nc = tc.nc
    KC, HC = K // P, H // P
    NB, JB, SC = SB // P, S // SB, S // P
    CTX_CH = [(o, min(512, K - o)) for o in range(0, K, 512)]
    TANH = mybir.ActivationFunctionType.Tanh
    EXP = mybir.ActivationFunctionType.Exp

    query = io["query"].ap()
    keys = io["keys"].ap()
    mask = io["mask"].ap()
    Wq = io["Wq"].ap()
    Wk = io["Wk"].ap()
    v = io["v"].ap()
    ctx_out = io["context"].ap()
    attn_out = io["attn"].ap()

    const = ctx.enter_context(tc.tile_pool(name="const", bufs=1))
    identB = const.tile([B, B], F32, name="identB")
    make_identity(nc, identB)
    ident1 = const.tile([1, 1], F32, name="ident1")
    nc.vector.memset(ident1, 1.0)
    neg_big = const.tile([1, SB], F32, name="neg_big")
    nc.vector.memset(neg_big, NEG_BIG)

    vT = const.tile([P, HC], F32, name="vT")
    nc.gpsimd.dma_start(out=vT, in_=v.rearrange("o (hc p) -> p (o hc)", p=P))
    vTb = const.tile([P, HC], BF16, name="vTb")
    nc.vector.tensor_copy(vTb, vT)
    qpT = const.tile([P, HC, B], F32, name="qpT")
    wkT4 = const.tile([P, HC, KC, P], BF16, name="wkT4")

    # ---------- prep: transpose weights via x-bar DMA, project query ----
    # Weights are cast-loaded to bf16 on SWDGE and transposed with single
    # x-bar DMAs on the (idle at this point) Scalar HWDGE queue; the keys
    # pipeline owns the Sync queue. No PE/DVE involvement at all.
    with tc.tile_pool(name="prep", bufs=1) as prep, tc.tile_pool(
        name="prep_ps", bufs=4, space=bass.MemorySpace.PSUM
    ) as prep_ps:
        q_nat = prep.tile([B, K], F32, name="q_nat")
        nc.sync.dma_start(out=q_nat, in_=query)
        wknb = prep.tile([P, HC, K], BF16, name="wknb")
        nc.gpsimd.dma_start(
            out=wknb, in_=Wk.rearrange("(hc p) k -> p hc k", p=P)
        )
        wqnb = prep.tile([P, HC, K], BF16, name="wqnb")
        nc.gpsimd.dma_start(
            out=wqnb, in_=Wq.rearrange("(hc p) k -> p hc k", p=P)
        )
        nc.scalar.dma_start_transpose(out=wkT4, in_=wknb)
        wqT4 = prep.tile([P, HC, KC, P], BF16, name="wqT4")
        nc.scalar.dma_start_transpose(out=wqT4, in_=wqnb)

        qT = prep.tile([P, KC, B], BF16, name="qT")
        psq = prep_ps.tile([P, KC, B], F32, name="psq", tag="pp")
        for kc in range(KC):
            nc.tensor.transpose(
                psq[:, kc, :], q_nat[:, kc * P : (kc + 1) * P], identB
            )
        nc.vector.tensor_copy(qT, psq)

        for hc in range(HC):
            psqp = prep_ps.tile([P, B], F32, name="psqp", tag="pp")
            for kc in range(KC):
                nc.tensor.matmul(
                    psqp,
                    wqT4[:, hc, kc, :],
                    qT[:, kc, :],
                    start=(kc == 0),
                    stop=(kc == KC - 1),
                )
            nc.vector.tensor_copy(qpT[:, hc, :], psqp)

    # ---------- main pools ----------
    knp = ctx.enter_context(tc.tile_pool(name="knp", bufs=2))
    knbp = ctx.enter_context(tc.tile_pool(name="knbp", bufs=7))
    ktp = ctx.enter_context(tc.tile_pool(name="ktp", bufs=3))
    thp = ctx.enter_context(tc.tile_pool(name="thp", bufs=9))
    stg = ctx.enter_context(tc.tile_pool(name="stg", bufs=2))
    atp = ctx.enter_context(tc.tile_pool(name="atp", bufs=2))
    PSUM = bass.MemorySpace.PSUM
    ps_kp = ctx.enter_context(tc.tile_pool(name="ps_kp", bufs=3, space=PSUM))
    ps_e = ctx.enter_context(tc.tile_pool(name="ps_e", bufs=2, space=PSUM))
    ps_c = ctx.enter_context(tc.tile_pool(name="ps_c", bufs=2, space=PSUM))

    def start_block(b, j):
        """HWDGE fp32 load -> DVE cast to bf16 -> x-bar DMA transpose."""
        kn = knp.tile([P, NB, K], F32, name="kn", tag="kn")
        nc.sync.dma_start(
            out=kn,
            in_=keys[b, j * SB : (j + 1) * SB, :].rearrange(
                "(nb p) k -> p nb k", p=P
            ),
        )
        knb = knbp.tile([P, NB, K], BF16, name="knb", tag="knb")
        nc.vector.tensor_copy(knb, kn)
        kT = ktp.tile([P, NB, KC, P], BF16, name="kT", tag="kT")
        nc.sync.dma_start_transpose(out=kT, in_=knb)
        return kT, knb

    # Energy matmuls are M=1; pack 4 of them into one PE pass on disjoint
    # 32-col groups (tile_position), giving 4 partial rows at partitions
    # 0/32/64/96 that GpSimd later all-reduces.
    EG = min(4, HC)  # energy col-pack width
    ER = (HC + EG - 1) // EG  # accumulation rounds per position

    def energy_pack(pe4, ths, r):
        for jj, (th, hc) in enumerate(ths):
            nc.tensor.matmul(
                pe4[32 * jj : 32 * jj + 1, :],
                vTb[:, hc : hc + 1],
                th,
                start=(r == 0),
                stop=(r == ER - 1),
                skip_group_check=True,
                tile_position=(0, 32 * jj),
            )

    def evac_energy(pe4, e_row4, b, j):
        for jj in range(EG):
            nc.scalar.copy(
                e_row4[32 * jj : 32 * jj + 1, j * SB : (j + 1) * SB],
                pe4[32 * jj : 32 * jj + 1, :],
            )

    def compute_block(b, j, kT, e_row4, carry):
        pe4 = ps_e.tile([P, SB], F32, name="pe4", tag="e")
        ths = []
        r = 0
        for hc in range(HC):
            pk = ps_kp.tile([P, SB], F32, name="pk", tag="kp")
            for kc in range(KC):
                nc.tensor.matmul(
                    pk,
                    wkT4[:, hc, kc, :],
                    kT[:, :, kc, :],
                    start=(kc == 0),
                    stop=(kc == KC - 1),
                )
            th = thp.tile([P, SB], BF16, name="th", tag="th")
            nc.scalar.activation(th, pk, TANH, bias=qpT[:, hc, b : b + 1], scale=1.0)
            ths.append((th, hc))
            if carry is not None:
                # flush the previous packed group while tanh of this group
                # is still in flight on ScalarE
                flush_carry(carry)
                carry = None
            if len(ths) == EG:
                carry = (pe4, ths, r, (e_row4, b, j))
                ths = []
                r += 1
        return carry

    def flush_carry(carry):
        pe4, ths, r, evac_args = carry
        energy_pack(pe4, ths, r)
        if r == ER - 1:
            evac_energy(pe4, *evac_args)

    def softmax(b, e_row4, m_row):
        e_cmp = stg.tile([max(EG, 2), S], F32, name="e_cmp", tag="ecmp")
        nc.gpsimd.dma_start(
            out=e_cmp[0:EG, :],
            in_=e_row4.rearrange("(a q) s -> a q s", q=32)[0:EG, 0, :],
        )
        if EG > 1:
            nc.gpsimd.partition_all_reduce(
                e_cmp[0:EG, :],
                e_cmp[0:EG, :],
                channels=EG,
                reduce_op=bass_isa.ReduceOp.add,
            )
        e_row = e_cmp[0:1, :]
        for j in range(JB):
            nc.vector.copy_predicated(
                e_row[0:1, j * SB : (j + 1) * SB],
                m_row[0:1, j * SB : (j + 1) * SB],
                neg_big,
            )
        mx = stg.tile([1, 1], F32, name="mx", tag="s1")
        nc.vector.reduce_max(out=mx, in_=e_row, axis=mybir.AxisListType.X)
        nm = stg.tile([1, 1], F32, name="nm", tag="s2")
        nc.scalar.mul(nm, mx, -1.0)
        se = stg.tile([1, 1], F32, name="se", tag="s3")
        nc.scalar.activation(e_row, e_row, EXP, bias=nm, scale=1.0, accum_out=se)
        ri = stg.tile([1, 1], F32, name="ri", tag="s4")
        nc.vector.reciprocal(ri, se)
        # all-pad rows: max == NEG_BIG exactly -> zero the whole row
        pad = stg.tile([1, 1], F32, name="pad", tag="s5")
        nc.vector.tensor_scalar(
            out=pad, in0=nm, scalar1=-NEG_BIG, scalar2=None,
            op0=mybir.AluOpType.is_equal,
        )
        valid = stg.tile([1, 1], F32, name="valid", tag="s6")
        nc.vector.tensor_scalar(
            out=valid, in0=pad, scalar1=-1.0, scalar2=1.0,
            op0=mybir.AluOpType.mult, op1=mybir.AluOpType.add,
        )
        ri2 = stg.tile([1, 1], F32, name="ri2", tag="s7")
        nc.vector.tensor_mul(ri2, ri, valid)
        nc.vector.tensor_scalar_mul(e_row, e_row, ri2)
        nc.sync.dma_start(out=attn_out[b : b + 1, :], in_=e_row)
        return e_row

    def attn_transpose(b, a_row):
        at_ps = ps_e.tile([P, SC], F32, name="at_ps", tag="e")
        for sc in range(SC):
            nc.tensor.transpose(
                at_ps[:, sc : sc + 1],
                a_row[0:1, sc * P : (sc + 1) * P],
                ident1,
            )
        atT = atp.tile([P, SC], BF16, name="atT", tag="atT")
        nc.vector.tensor_copy(atT, at_ps)
        return atT

    CG = min(4, SC)  # context col-pack width
    CR = SC // CG

    def context_pass(b, atT, knbs_b):
        pc4s = [
            ps_c.tile([P, chn], F32, name=f"pc4_{i}", tag="c")
            for i, (o, chn) in enumerate(CTX_CH)
        ]
        for rr in range(CR):
            for i, (o, chn) in enumerate(CTX_CH):
                for jj in range(CG):
                    sc = rr * CG + jj
                    j2, nb = divmod(sc, NB)
                    nc.tensor.matmul(
                        pc4s[i][32 * jj : 32 * jj + 1, :],
                        atT[:, sc : sc + 1],
                        knbs_b[j2][:, nb, o : o + chn],
                        start=(rr == 0),
                        stop=(rr == CR - 1),
                        skip_group_check=True,
                        tile_position=(0, 32 * jj),
                    )
        ctx4 = stg.tile([P, K], F32, name="ctx4", tag="crow")
        for i, (o, chn) in enumerate(CTX_CH):
            for jj in range(CG):
                nc.scalar.copy(
                    ctx4[32 * jj : 32 * jj + 1, o : o + chn],
                    pc4s[i][32 * jj : 32 * jj + 1, :],
                )
        c_cmp = stg.tile([max(CG, 2), K], F32, name="c_cmp", tag="ccmp")
        nc.gpsimd.dma_start(
            out=c_cmp[0:CG, :],
            in_=ctx4.rearrange("(a q) k -> a q k", q=32)[0:CG, 0, :],
        )
        if CG > 1:
            nc.gpsimd.partition_all_reduce(
                c_cmp[0:CG, :],
                c_cmp[0:CG, :],
                channels=CG,
                reduce_op=bass_isa.ReduceOp.add,
            )
        nc.sync.dma_start(out=ctx_out[b : b + 1, :], in_=c_cmp[0:1, :])

    # ---------- main schedule ----------
    def new_b_tiles(b):
        e = stg.tile([P, S], F32, name="e_row4", tag="erow")
        m = stg.tile([1, S], U8, name="m_row", tag="mrow")
        nc.sync.dma_start(out=m, in_=mask[b : b + 1, :])
        return e, m

    blocks = [(b, j) for b in range(B) for j in range(JB)]
    LOOKAHEAD = 2
    e_rows, m_rows, a_rows = {}, {}, {}
    knbs, kts = {}, {}
    e_rows[0], m_rows[0] = new_b_tiles(0)
    for i in range(min(LOOKAHEAD, len(blocks))):
        kts[blocks[i]], knbs[blocks[i]] = start_block(*blocks[i])
    carry = None
    for idx, (b, j) in enumerate(blocks):
        if idx + LOOKAHEAD < len(blocks):
            nxt_blk = blocks[idx + LOOKAHEAD]
            kts[nxt_blk], knbs[nxt_blk] = start_block(*nxt_blk)
        carry = compute_block(b, j, kts.pop((b, j)), e_rows[b], carry)
        if j == 0 and b > 0:
            context_pass(
                b - 1,
                attn_transpose(b - 1, a_rows[b - 1]),
                [knbs.pop((b - 1, jj)) for jj in range(JB)],
            )
        if j == JB - 1:
            if carry is not None:
                flush_carry(carry)
                carry = None
            a_rows[b] = softmax(b, e_rows[b], m_rows[b])
            if b + 1 < B:
                e_rows[b + 1], m_rows[b + 1] = new_b_tiles(b + 1)
    context_pass(
        B - 1,
        attn_transpose(B - 1, a_rows[B - 1]),
        [knbs.pop((B - 1, jj)) for jj in range(JB)],
    )


def build_kernel(B=FULL_B // N_CORES, S=FULL_S, K=FULL_K, H=FULL_H, SB=512):
    nc = bacc.Bacc(
        "TRN2", target_bir_lowering=False, debug=False, enable_partition_id=False
    )
    io = {
        "query": nc.dram_tensor("query", [B, K], F32, kind="ExternalInput"),
        "keys": nc.dram_tensor("keys", [B, S, K], F32, kind="ExternalInput"),
        "mask": nc.dram_tensor("mask", [B, S], U8, kind="ExternalInput"),
        "Wq": nc.dram_tensor("Wq", [H, K], F32, kind="ExternalInput"),
        "Wk": nc.dram_tensor("Wk", [H, K], F32, kind="ExternalInput"),
        "v": nc.dram_tensor("v", [1, H], F32, kind="ExternalInput"),
        "context": nc.dram_tensor("context", [B, K], F32, kind="ExternalOutput"),
        "attn": nc.dram_tensor("attn", [B, S], F32, kind="ExternalOutput"),
    }
    with tile.TileContext(nc) as tc:
        with ExitStack() as ctx:
            _emit(ctx, tc, io, B, S, K, H, SB)
    nc.compile()
    return nc


def ref_np(query, keys, mask, Wq, Wk, v):
    """Numpy reference (mirrors the jax oracle) for dev testing."""
    qp = query.astype(np.float64) @ Wq.T.astype(np.float64)
    kp = np.einsum("bsk,hk->bsh", keys, Wk, dtype=np.float64)
    sc = np.tanh(qp[:, None, :] + kp)
    en = np.einsum("bsh,h->bs", sc, v[0].astype(np.float64))
    en = np.where(mask, -np.inf, en)
    mx = np.max(en, axis=-1, keepdims=True)
    mx = np.where(np.isfinite(mx), mx, 0.0)
    ex = np.exp(en - mx)
    sm = ex.sum(axis=-1, keepdims=True)
    attn = np.where(sm > 0, ex / np.where(sm == 0, 1.0, sm), 0.0)
    ctxo = np.einsum("bs,bsk->bk", attn, keys, dtype=np.float64)
    return ctxo.astype(np.float32), attn.astype(np.float32)


_CACHE = {}


def _get_nc():
    if "nc" not in _CACHE:
        _CACHE["nc"] = build_kernel()
    return _CACHE["nc"]


def kernel(query, keys, mask, Wq, Wk, v):
    query = np.ascontiguousarray(np.asarray(query), dtype=np.float32)
    keys = np.ascontiguousarray(np.asarray(keys), dtype=np.float32)
    mask_u8 = np.ascontiguousarray(np.asarray(mask)).astype(np.uint8)
    Wq = np.ascontiguousarray(np.asarray(Wq), dtype=np.float32)
    Wk = np.ascontiguousarray(np.asarray(Wk), dtype=np.float32)
    v = np.ascontiguousarray(np.asarray(v), dtype=np.float32)

    nc = _get_nc()
    bs = FULL_B // N_CORES
    in_maps = []
    for c in range(N_CORES):
        sl = slice(c * bs, (c + 1) * bs)
        in_maps.append(
            {
                "query": np.ascontiguousarray(query[sl]),
                "keys": np.ascontiguousarray(keys[sl]),
                "mask": np.ascontiguousarray(mask_u8[sl]),
                "Wq": Wq,
                "Wk": Wk,
                "v": v,
            }
        )
    res = bass_utils.run_bass_kernel_spmd(nc, in_maps, core_ids=list(range(N_CORES)))
    context = np.concatenate([r["context"] for r in res.results], axis=0)
    attn = np.concatenate([r["attn"] for r in res.results], axis=0)
    return context, attn


# revision 54
# speedup vs baseline: 1.2912x; 1.2912x over previous
"""Bahdanau additive attention on TRN2 (Bass/Tile), 8-core data-parallel.

Math (per batch row b):
    qp   = query @ Wq.T                      # [H]
    kp_s = keys[s] @ Wk.T                    # [S, H]
    e_s  = v . tanh(qp + kp_s)               # [S]
    attn = softmax(mask(e))                  # [S]  (all-pad rows -> 0)
    ctx  = attn @ keys                       # [K]

Sharding: B=64 batches split 8 ways (8 per core); weights replicated.

Per-core kernel layout strategy:
  - The projection contracts over k, so keys tiles are PE-transposed
    ([s,k] -> [k,s]) and the matmul runs with Wk^T chunks stationary,
    producing kp^T [h, s] in PSUM (fp32r matmuls: full-rate with fp32
    storage).
  - tanh(kp^T + qp^T) is fused on ScalarE via activation bias (per
    partition = per h).
  - energy = v . tanh(...) is a second PE matmul contracting h.
  - softmax per batch row runs on partition 0 (tiny: [1, S]).
  - context re-streams keys in natural [s, k] layout and contracts s
    with the attention column as the stationary operand.
"""

import os
import sys

import numpy as np

for _p in ("/opt/trn_rl_repo", os.path.expanduser("~/.axon_site/_ro/trn_rl_repo")):
    if os.path.isdir(_p) and _p not in sys.path:
        sys.path.append(_p)

from contextlib import ExitStack

import concourse.bacc as bacc
import concourse.bass as bass
import concourse.mybir as mybir
import concourse.tile as tile
from concourse import bass_isa, bass_utils
from concourse.masks import make_identity

F32 = mybir.dt.float32
F32R = mybir.dt.float32r
BF16 = mybir.dt.bfloat16
U8 = mybir.dt.uint8
P = 128

N_CORES = 8
FULL_B, FULL_S, FULL_H, FULL_K = 64, 2048, 1024, 1024

NEG_BIG = -30.0  # masked-energy fill; |energy| << 30 so never collides


def _emit(ctx, tc, io, B, S, K, H, SB):
    nc = tc.nc
    KC, HC = K // P, H // P
    NB, JB, SC = SB // P, S // SB, S // P
    CTX_CH = [(o, min(512, K - o)) for o in range(0, K, 512)]
    TANH = mybir.ActivationFunctionType.Tanh
    EXP = mybir.ActivationFunctionType.Exp

    query = io["query"].ap()
    keys = io["keys"].ap()
    mask = io["mask"].ap()
    Wq = io["Wq"].ap()
    Wk = io["Wk"].ap()
    v = io["v"].ap()
    ctx_out = io["context"].ap()
    attn_out = io["attn"].ap()

    const = ctx.enter_context(tc.tile_pool(name="const", bufs=1))
    identB = const.tile([B, B], F32, name="identB")
    make_identity(nc, identB)
    ident1 = const.tile([1, 1], F32, name="ident1")
    nc.vector.memset(ident1, 1.0)
    neg_big = const.tile([1, SB], F32, name="neg_big")
    nc.vector.memset(neg_big, NEG_BIG)

    vT = const.tile([P, HC], F32, name="vT")
    nc.gpsimd.dma_start(out=vT, in_=v.rearrange("o (hc p) -> p (o hc)", p=P))
    vTb = const.tile([P, HC], BF16, name="vTb")
    nc.vector.tensor_copy(vTb, vT)
    qpT = const.tile([P, HC, B], F32, name="qpT")
    wkT4 = const.tile([P, HC, KC, P], BF16, name="wkT4")

    # ---------- prep: transpose weights via x-bar DMA, project query ----
    # Weights are cast-loaded to bf16 on SWDGE and transposed with single
    # x-bar DMAs on the (idle at this point) Scalar HWDGE queue; the keys
    # pipeline owns the Sync queue. No PE/DVE involvement at all.
    with tc.tile_pool(name="prep", bufs=1) as prep, tc.tile_pool(
        name="prep_ps", bufs=4, space=bass.MemorySpace.PSUM
    ) as prep_ps:
        q_nat = prep.tile([B, K], F32, name="q_nat")
        nc.sync.dma_start(out=q_nat, in_=query)
        wknb = prep.tile([P, HC, K], BF16, name="wknb")
        nc.gpsimd.dma_start(
            out=wknb, in_=Wk.rearrange("(hc p) k -> p hc k", p=P)
        )
        wqnb = prep.tile([P, HC, K], BF16, name="wqnb")
        nc.gpsimd.dma_start(
            out=wqnb, in_=Wq.rearrange("(hc p) k -> p hc k", p=P)
        )
        nc.scalar.dma_start_transpose(out=wkT4, in_=wknb)
        wqT4 = prep.tile([P, HC, KC, P], BF16, name="wqT4")
        nc.scalar.dma_start_transpose(out=wqT4, in_=wqnb)

        qT = prep.tile([P, KC, B], BF16, name="qT")
        psq = prep_ps.tile([P, KC, B], F32, name="psq", tag="pp")
        for kc in range(KC):
            nc.tensor.transpose(
                psq[:, kc, :], q_nat[:, kc * P : (kc + 1) * P], identB
            )
        nc.vector.tensor_copy(qT, psq)

        for hc in range(HC):
            psqp = prep_ps.tile([P, B], F32, name="psqp", tag="pp")
            for kc in range(KC):
                nc.tensor.matmul(
                    psqp,
                    wqT4[:, hc, kc, :],
                    qT[:, kc, :],
                    start=(kc == 0),
                    stop=(kc == KC - 1),
                )
            nc.vector.tensor_copy(qpT[:, hc, :], psqp)

    # ---------- main pools ----------
    knp = ctx.enter_context(tc.tile_pool(name="knp", bufs=2))
    knbp = ctx.enter_context(tc.tile_pool(name="knbp", bufs=7))
    ktp = ctx.enter_context(tc.tile_pool(name="ktp", bufs=3))
    thp = ctx.enter_context(tc.tile_pool(name="thp", bufs=9))
    stg = ctx.enter_context(tc.tile_pool(name="stg", bufs=2))
    atp = ctx.enter_context(tc.tile_pool(name="atp", bufs=2))
    PSUM = bass.MemorySpace.PSUM
    ps_kp = ctx.enter_context(tc.tile_pool(name="ps_kp", bufs=3, space=PSUM))
    ps_e = ctx.enter_context(tc.tile_pool(name="ps_e", bufs=2, space=PSUM))
    ps_c = ctx.enter_context(tc.tile_pool(name="ps_c", bufs=2, space=PSUM))

    def start_block(b, j):
        """HWDGE fp32 load -> DVE cast to bf16 -> x-bar DMA transpose."""
        kn = knp.tile([P, NB, K], F32, name="kn", tag="kn")
        nc.sync.dma_start(
            out=kn,
            in_=keys[b, j * SB : (j + 1) * SB, :].rearrange(
                "(nb p) k -> p nb k", p=P
            ),
        )
        knb = knbp.tile([P, NB, K], BF16, name="knb", tag="knb")
        nc.vector.tensor_copy(knb, kn)
        kT = ktp.tile([P, NB, KC, P], BF16, name="kT", tag="kT")
        nc.sync.dma_start_transpose(out=kT, in_=knb)
        return kT, knb

    # Energy matmuls are M=1; pack 4 of them into one PE pass on disjoint
    # 32-col groups (tile_position), giving 4 partial rows at partitions
    # 0/32/64/96 that GpSimd later all-reduces.
    EG = min(4, HC)  # energy col-pack width
    ER = (HC + EG - 1) // EG  # accumulation rounds per position

    def energy_pack(pe4, ths, r):
        for jj, (th, hc) in enumerate(ths):
            nc.tensor.matmul(
                pe4[32 * jj : 32 * jj + 1, :],
                vTb[:, hc : hc + 1],
                th,
                start=(r == 0),
                stop=(r == ER - 1),
                skip_group_check=True,
                tile_position=(0, 32 * jj),
            )

    def evac_energy(pe4, e_row4, b, j):
        sl = e_row4[0:1, j * SB : (j + 1) * SB]
        nc.scalar.copy(sl, pe4[0:1, :])
        for jj in range(1, EG):
            nc.vector.tensor_add(sl, sl, pe4[32 * jj : 32 * jj + 1, :])

    def compute_block(b, j, kT, e_row4, carry):
        pe4 = ps_e.tile([P, SB], F32, name="pe4", tag="e")
        ths = []
        r = 0
        for hc in range(HC):
            pk = ps_kp.tile([P, SB], F32, name="pk", tag="kp")
            for kc in range(KC):
                nc.tensor.matmul(
                    pk,
                    wkT4[:, hc, kc, :],
                    kT[:, :, kc, :],
                    start=(kc == 0),
                    stop=(kc == KC - 1),
                )
            th = thp.tile([P, SB], BF16, name="th", tag="th")
            nc.scalar.activation(th, pk, TANH, bias=qpT[:, hc, b : b + 1], scale=1.0)
            ths.append((th, hc))
            if carry is not None:
                # flush the previous packed group while tanh of this group
                # is still in flight on ScalarE
                flush_carry(carry)
                carry = None
            if len(ths) == EG:
                carry = (pe4, ths, r, (e_row4, b, j))
                ths = []
                r += 1
        return carry

    def flush_carry(carry):
        pe4, ths, r, evac_args = carry
        energy_pack(pe4, ths, r)
        if r == ER - 1:
            evac_energy(pe4, *evac_args)

    def softmax(b, e_row4, m_row):
        e_row = e_row4[0:1, :]
        for j in range(JB):
            nc.vector.copy_predicated(
                e_row[0:1, j * SB : (j + 1) * SB],
                m_row[0:1, j * SB : (j + 1) * SB],
                neg_big,
            )
        mx = stg.tile([1, 1], F32, name="mx", tag="s1")
        nc.vector.reduce_max(out=mx, in_=e_row, axis=mybir.AxisListType.X)
        nm = stg.tile([1, 1], F32, name="nm", tag="s2")
        nc.scalar.mul(nm, mx, -1.0)
        se = stg.tile([1, 1], F32, name="se", tag="s3")
        nc.scalar.activation(e_row, e_row, EXP, bias=nm, scale=1.0, accum_out=se)
        ri = stg.tile([1, 1], F32, name="ri", tag="s4")
        nc.vector.reciprocal(ri, se)
        # all-pad rows: max == NEG_BIG exactly -> zero the whole row
        pad = stg.tile([1, 1], F32, name="pad", tag="s5")
        nc.vector.tensor_scalar(
            out=pad, in0=nm, scalar1=-NEG_BIG, scalar2=None,
            op0=mybir.AluOpType.is_equal,
        )
        valid = stg.tile([1, 1], F32, name="valid", tag="s6")
        nc.vector.tensor_scalar(
            out=valid, in0=pad, scalar1=-1.0, scalar2=1.0,
            op0=mybir.AluOpType.mult, op1=mybir.AluOpType.add,
        )
        ri2 = stg.tile([1, 1], F32, name="ri2", tag="s7")
        nc.vector.tensor_mul(ri2, ri, valid)
        nc.vector.tensor_scalar_mul(e_row, e_row, ri2)
        nc.sync.dma_start(out=attn_out[b : b + 1, :], in_=e_row)
        return e_row

    def attn_transpose(b, a_row):
        at_ps = ps_e.tile([P, SC], F32, name="at_ps", tag="e")
        for sc in range(SC):
            nc.tensor.transpose(
                at_ps[:, sc : sc + 1],
                a_row[0:1, sc * P : (sc + 1) * P],
                ident1,
            )
        atT = atp.tile([P, SC], BF16, name="atT", tag="atT")
        nc.vector.tensor_copy(atT, at_ps)
        return atT

    CG = min(4, SC)  # context col-pack width
    CR = SC // CG

    def context_pass(b, atT, knbs_b):
        pc4s = [
            ps_c.tile([P, chn], F32, name=f"pc4_{i}", tag="c")
            for i, (o, chn) in enumerate(CTX_CH)
        ]
        for rr in range(CR):
            for i, (o, chn) in enumerate(CTX_CH):
                for jj in range(CG):
                    sc = rr * CG + jj
                    j2, nb = divmod(sc, NB)
                    nc.tensor.matmul(
                        pc4s[i][32 * jj : 32 * jj + 1, :],
                        atT[:, sc : sc + 1],
                        knbs_b[j2][:, nb, o : o + chn],
                        start=(rr == 0),
                        stop=(rr == CR - 1),
                        skip_group_check=True,
                        tile_position=(0, 32 * jj),
                    )
        ctx4 = stg.tile([1, K], F32, name="ctx4", tag="crow")
        for i, (o, chn) in enumerate(CTX_CH):
            sl = ctx4[0:1, o : o + chn]
            nc.scalar.copy(sl, pc4s[i][0:1, :])
            for jj in range(1, CG):
                nc.vector.tensor_add(sl, sl, pc4s[i][32 * jj : 32 * jj + 1, :])
        nc.sync.dma_start(out=ctx_out[b : b + 1, :], in_=ctx4[0:1, :])

    # ---------- main schedule ----------
    def new_b_tiles(b):
        e = stg.tile([1, S], F32, name="e_row4", tag="erow")
        m = stg.tile([1, S], U8, name="m_row", tag="mrow")
        nc.sync.dma_start(out=m, in_=mask[b : b + 1, :])
        return e, m

    blocks = [(b, j) for b in range(B) for j in range(JB)]
    LOOKAHEAD = 2
    e_rows, m_rows, a_rows = {}, {}, {}
    knbs, kts = {}, {}
    e_rows[0], m_rows[0] = new_b_tiles(0)
    for i in range(min(LOOKAHEAD, len(blocks))):
        kts[blocks[i]], knbs[blocks[i]] = start_block(*blocks[i])
    carry = None
    for idx, (b, j) in enumerate(blocks):
        if idx + LOOKAHEAD < len(blocks):
            nxt_blk = blocks[idx + LOOKAHEAD]
            kts[nxt_blk], knbs[nxt_blk] = start_block(*nxt_blk)
        carry = compute_block(b, j, kts.pop((b, j)), e_rows[b], carry)
        if j == 0 and b > 0:
            context_pass(
                b - 1,
                attn_transpose(b - 1, a_rows[b - 1]),
                [knbs.pop((b - 1, jj)) for jj in range(JB)],
            )
        if j == JB - 1:
            if carry is not None:
                flush_carry(carry)
                carry = None
            a_rows[b] = softmax(b, e_rows[b], m_rows[b])
            if b + 1 < B:
                e_rows[b + 1], m_rows[b + 1] = new_b_tiles(b + 1)
    context_pass(
        B - 1,
        attn_transpose(B - 1, a_rows[B - 1]),
        [knbs.pop((B - 1, jj)) for jj in range(JB)],
    )


def build_kernel(B=FULL_B // N_CORES, S=FULL_S, K=FULL_K, H=FULL_H, SB=512):
    nc = bacc.Bacc(
        "TRN2", target_bir_lowering=False, debug=False, enable_partition_id=False
    )
    io = {
        "query": nc.dram_tensor("query", [B, K], F32, kind="ExternalInput"),
        "keys": nc.dram_tensor("keys", [B, S, K], F32, kind="ExternalInput"),
        "mask": nc.dram_tensor("mask", [B, S], U8, kind="ExternalInput"),
        "Wq": nc.dram_tensor("Wq", [H, K], F32, kind="ExternalInput"),
        "Wk": nc.dram_tensor("Wk", [H, K], F32, kind="ExternalInput"),
        "v": nc.dram_tensor("v", [1, H], F32, kind="ExternalInput"),
        "context": nc.dram_tensor("context", [B, K], F32, kind="ExternalOutput"),
        "attn": nc.dram_tensor("attn", [B, S], F32, kind="ExternalOutput"),
    }
    with tile.TileContext(nc) as tc:
        with ExitStack() as ctx:
            _emit(ctx, tc, io, B, S, K, H, SB)
    nc.compile()
    return nc


def ref_np(query, keys, mask, Wq, Wk, v):
    """Numpy reference (mirrors the jax oracle) for dev testing."""
    qp = query.astype(np.float64) @ Wq.T.astype(np.float64)
    kp = np.einsum("bsk,hk->bsh", keys, Wk, dtype=np.float64)
    sc = np.tanh(qp[:, None, :] + kp)
    en = np.einsum("bsh,h->bs", sc, v[0].astype(np.float64))
    en = np.where(mask, -np.inf, en)
    mx = np.max(en, axis=-1, keepdims=True)
    mx = np.where(np.isfinite(mx), mx, 0.0)
    ex = np.exp(en - mx)
    sm = ex.sum(axis=-1, keepdims=True)
    attn = np.where(sm > 0, ex / np.where(sm == 0, 1.0, sm), 0.0)
    ctxo = np.einsum("bs,bsk->bk", attn, keys, dtype=np.float64)
    return ctxo.astype(np.float32), attn.astype(np.float32)


_CACHE = {}


def _get_nc():
    if "nc" not in _CACHE:
        _CACHE["nc"] = build_kernel()
    return _CACHE["nc"]


def kernel(query, keys, mask, Wq, Wk, v):
    query = np.ascontiguousarray(np.asarray(query), dtype=np.float32)
    keys = np.ascontiguousarray(np.asarray(keys), dtype=np.float32)
    mask_u8 = np.ascontiguousarray(np.asarray(mask)).astype(np.uint8)
    Wq = np.ascontiguousarray(np.asarray(Wq), dtype=np.float32)
    Wk = np.ascontiguousarray(np.asarray(Wk), dtype=np.float32)
    v = np.ascontiguousarray(np.asarray(v), dtype=np.float32)

    nc = _get_nc()
    bs = FULL_B // N_CORES
    in_maps = []
    for c in range(N_CORES):
        sl = slice(c * bs, (c + 1) * bs)
        in_maps.append(
            {
                "query": np.ascontiguousarray(query[sl]),
                "keys": np.ascontiguousarray(keys[sl]),
                "mask": np.ascontiguousarray(mask_u8[sl]),
                "Wq": Wq,
                "Wk": Wk,
                "v": v,
            }
        )
    res = bass_utils.run_bass_kernel_spmd(nc, in_maps, core_ids=list(range(N_CORES)))
    context = np.concatenate([r["context"] for r in res.results], axis=0)
    attn = np.concatenate([r["attn"] for r in res.results], axis=0)
    return context, attn


# revision 57
# speedup vs baseline: 1.3158x; 1.0191x over previous
"""Bahdanau additive attention on TRN2 (Bass/Tile), 8-core data-parallel.

Math (per batch row b):
    qp   = query @ Wq.T                      # [H]
    kp_s = keys[s] @ Wk.T                    # [S, H]
    e_s  = v . tanh(qp + kp_s)               # [S]
    attn = softmax(mask(e))                  # [S]  (all-pad rows -> 0)
    ctx  = attn @ keys                       # [K]

Sharding: B=64 batches split 8 ways (8 per core); weights replicated.

Per-core kernel layout strategy:
  - The projection contracts over k, so keys tiles are PE-transposed
    ([s,k] -> [k,s]) and the matmul runs with Wk^T chunks stationary,
    producing kp^T [h, s] in PSUM (fp32r matmuls: full-rate with fp32
    storage).
  - tanh(kp^T + qp^T) is fused on ScalarE via activation bias (per
    partition = per h).
  - energy = v . tanh(...) is a second PE matmul contracting h.
  - softmax per batch row runs on partition 0 (tiny: [1, S]).
  - context re-streams keys in natural [s, k] layout and contracts s
    with the attention column as the stationary operand.
"""

import os
import sys

import numpy as np

for _p in ("/opt/trn_rl_repo", os.path.expanduser("~/.axon_site/_ro/trn_rl_repo")):
    if os.path.isdir(_p) and _p not in sys.path:
        sys.path.append(_p)

from contextlib import ExitStack

import concourse.bacc as bacc
import concourse.bass as bass
import concourse.mybir as mybir
import concourse.tile as tile
from concourse import bass_isa, bass_utils
from concourse.masks import make_identity

F32 = mybir.dt.float32
F32R = mybir.dt.float32r
BF16 = mybir.dt.bfloat16
U8 = mybir.dt.uint8
P = 128

N_CORES = 8
FULL_B, FULL_S, FULL_H, FULL_K = 64, 2048, 1024, 1024

NEG_BIG = -30.0  # masked-energy fill; |energy| << 30 so never collides


def _emit(ctx, tc, io, B, S, K, H, SB):
    nc = tc.nc
    KC, HC = K // P, H // P
    NB, JB, SC = SB // P, S // SB, S // P
    CTX_CH = [(o, min(512, K - o)) for o in range(0, K, 512)]
    TANH = mybir.ActivationFunctionType.Tanh
    EXP = mybir.ActivationFunctionType.Exp

    query = io["query"].ap()
    keys = io["keys"].ap()
    mask = io["mask"].ap()
    Wq = io["Wq"].ap()
    Wk = io["Wk"].ap()
    v = io["v"].ap()
    ctx_out = io["context"].ap()
    attn_out = io["attn"].ap()

    const = ctx.enter_context(tc.tile_pool(name="const", bufs=1))
    qpT = const.tile([P, HC, B], F32, name="qpT")
    wkT4 = const.tile([P, HC, KC, P], BF16, name="wkT4")

    # ---------- prep: x-bar weight transposes + query projection --------
    with tc.tile_pool(name="prep", bufs=1) as prep, tc.tile_pool(
        name="prep_ps", bufs=4, space=bass.MemorySpace.PSUM
    ) as prep_ps:
        # Weight pipeline first: SWDGE cast-loads (gpsimd queue is
        # otherwise empty) feeding x-bar transposes on the Scalar HWDGE
        # queue. Wq goes first (the query projection gates the first
        # tanh); Wk is split in halves so the first projection matmuls
        # can start sooner.
        wqnb = prep.tile([P, HC, K], BF16, name="wqnb")
        nc.gpsimd.dma_start(
            out=wqnb, in_=Wq.rearrange("(hc p) k -> p hc k", p=P)
        )
        wknb = prep.tile([P, HC, K], BF16, name="wknb")
        wk_src = Wk.rearrange("(hc p) k -> p hc k", p=P)
        H2 = HC // 2 if HC > 1 else HC
        nc.gpsimd.dma_start(out=wknb[:, 0:H2, :], in_=wk_src[:, 0:H2, :])
        if H2 < HC:
            nc.gpsimd.dma_start(out=wknb[:, H2:HC, :], in_=wk_src[:, H2:HC, :])
        wqT4 = prep.tile([P, HC, KC, P], BF16, name="wqT4")
        nc.scalar.dma_start_transpose(out=wqT4, in_=wqnb)
        nc.scalar.dma_start_transpose(
            out=wkT4[:, 0:H2, :, :], in_=wknb[:, 0:H2, :]
        )
        if H2 < HC:
            nc.scalar.dma_start_transpose(
                out=wkT4[:, H2:HC, :, :], in_=wknb[:, H2:HC, :]
            )

        q_nat = prep.tile([B, K], F32, name="q_nat")
        nc.sync.dma_start(out=q_nat, in_=query)

        identB = const.tile([B, B], F32, name="identB")
        make_identity(nc, identB)
        ident1 = const.tile([1, 1], F32, name="ident1")
        nc.vector.memset(ident1, 1.0)
        neg_big = const.tile([1, SB], F32, name="neg_big")
        nc.vector.memset(neg_big, NEG_BIG)
        vT = const.tile([P, HC], F32, name="vT")
        nc.gpsimd.dma_start(out=vT, in_=v.rearrange("o (hc p) -> p (o hc)", p=P))
        vTb = const.tile([P, HC], BF16, name="vTb")
        nc.vector.tensor_copy(vTb, vT)

        qT = prep.tile([P, KC, B], BF16, name="qT")
        psq = prep_ps.tile([P, KC, B], F32, name="psq", tag="pp")
        for kc in range(KC):
            nc.tensor.transpose(
                psq[:, kc, :], q_nat[:, kc * P : (kc + 1) * P], identB
            )
        nc.vector.tensor_copy(qT, psq)

        for hc in range(HC):
            psqp = prep_ps.tile([P, B], F32, name="psqp", tag="pp")
            for kc in range(KC):
                nc.tensor.matmul(
                    psqp,
                    wqT4[:, hc, kc, :],
                    qT[:, kc, :],
                    start=(kc == 0),
                    stop=(kc == KC - 1),
                )
            nc.vector.tensor_copy(qpT[:, hc, :], psqp)

    # ---------- main pools ----------
    knp = ctx.enter_context(tc.tile_pool(name="knp", bufs=2))
    knbp = ctx.enter_context(tc.tile_pool(name="knbp", bufs=7))
    ktp = ctx.enter_context(tc.tile_pool(name="ktp", bufs=3))
    thp = ctx.enter_context(tc.tile_pool(name="thp", bufs=9))
    stg = ctx.enter_context(tc.tile_pool(name="stg", bufs=2))
    atp = ctx.enter_context(tc.tile_pool(name="atp", bufs=2))
    PSUM = bass.MemorySpace.PSUM
    ps_kp = ctx.enter_context(tc.tile_pool(name="ps_kp", bufs=3, space=PSUM))
    ps_e = ctx.enter_context(tc.tile_pool(name="ps_e", bufs=2, space=PSUM))
    ps_c = ctx.enter_context(tc.tile_pool(name="ps_c", bufs=2, space=PSUM))

    def start_block(b, j):
        """HWDGE fp32 load -> DVE cast to bf16 -> x-bar DMA transpose."""
        kn = knp.tile([P, NB, K], F32, name="kn", tag="kn")
        nc.sync.dma_start(
            out=kn,
            in_=keys[b, j * SB : (j + 1) * SB, :].rearrange(
                "(nb p) k -> p nb k", p=P
            ),
        )
        knb = knbp.tile([P, NB, K], BF16, name="knb", tag="knb")
        nc.vector.tensor_copy(knb, kn)
        kT = ktp.tile([P, NB, KC, P], BF16, name="kT", tag="kT")
        nc.sync.dma_start_transpose(out=kT, in_=knb)
        return kT, knb

    # Energy matmuls are M=1; pack 4 of them into one PE pass on disjoint
    # 32-col groups (tile_position), giving 4 partial rows at partitions
    # 0/32/64/96 that GpSimd later all-reduces.
    EG = min(4, HC)  # energy col-pack width
    ER = (HC + EG - 1) // EG  # accumulation rounds per position

    def energy_pack(pe4, ths, r):
        for jj, (th, hc) in enumerate(ths):
            nc.tensor.matmul(
                pe4[32 * jj : 32 * jj + 1, :],
                vTb[:, hc : hc + 1],
                th,
                start=(r == 0),
                stop=(r == ER - 1),
                skip_group_check=True,
                tile_position=(0, 32 * jj),
            )

    def evac_energy(pe4, e_row4, b, j):
        sl = e_row4[0:1, j * SB : (j + 1) * SB]
        nc.scalar.copy(sl, pe4[0:1, :])
        for jj in range(1, EG):
            nc.vector.tensor_add(sl, sl, pe4[32 * jj : 32 * jj + 1, :])

    def compute_block(b, j, kT, e_row4, carry):
        pe4 = ps_e.tile([P, SB], F32, name="pe4", tag="e")
        ths = []
        r = 0
        for hc in range(HC):
            pk = ps_kp.tile([P, SB], F32, name="pk", tag="kp")
            for kc in range(KC):
                nc.tensor.matmul(
                    pk,
                    wkT4[:, hc, kc, :],
                    kT[:, :, kc, :],
                    start=(kc == 0),
                    stop=(kc == KC - 1),
                )
            th = thp.tile([P, SB], BF16, name="th", tag="th")
            nc.scalar.activation(th, pk, TANH, bias=qpT[:, hc, b : b + 1], scale=1.0)
            ths.append((th, hc))
            if carry is not None:
                # flush the previous packed group while tanh of this group
                # is still in flight on ScalarE
                flush_carry(carry)
                carry = None
            if len(ths) == EG:
                carry = (pe4, ths, r, (e_row4, b, j))
                ths = []
                r += 1
        return carry

    def flush_carry(carry):
        pe4, ths, r, evac_args = carry
        energy_pack(pe4, ths, r)
        if r == ER - 1:
            evac_energy(pe4, *evac_args)

    def softmax(b, e_row4, m_row):
        e_row = e_row4[0:1, :]
        for j in range(JB):
            nc.vector.copy_predicated(
                e_row[0:1, j * SB : (j + 1) * SB],
                m_row[0:1, j * SB : (j + 1) * SB],
                neg_big,
            )
        mx = stg.tile([1, 1], F32, name="mx", tag="s1")
        nc.vector.reduce_max(out=mx, in_=e_row, axis=mybir.AxisListType.X)
        nm = stg.tile([1, 1], F32, name="nm", tag="s2")
        nc.scalar.mul(nm, mx, -1.0)
        se = stg.tile([1, 1], F32, name="se", tag="s3")
        nc.scalar.activation(e_row, e_row, EXP, bias=nm, scale=1.0, accum_out=se)
        ri = stg.tile([1, 1], F32, name="ri", tag="s4")
        nc.vector.reciprocal(ri, se)
        # all-pad rows: max == NEG_BIG exactly -> zero the whole row
        pad = stg.tile([1, 1], F32, name="pad", tag="s5")
        nc.vector.tensor_scalar(
            out=pad, in0=nm, scalar1=-NEG_BIG, scalar2=None,
            op0=mybir.AluOpType.is_equal,
        )
        valid = stg.tile([1, 1], F32, name="valid", tag="s6")
        nc.vector.tensor_scalar(
            out=valid, in0=pad, scalar1=-1.0, scalar2=1.0,
            op0=mybir.AluOpType.mult, op1=mybir.AluOpType.add,
        )
        ri2 = stg.tile([1, 1], F32, name="ri2", tag="s7")
        nc.vector.tensor_mul(ri2, ri, valid)
        nc.vector.tensor_scalar_mul(e_row, e_row, ri2)
        nc.sync.dma_start(out=attn_out[b : b + 1, :], in_=e_row)
        return e_row

    def attn_transpose(b, a_row):
        at_ps = ps_e.tile([P, SC], F32, name="at_ps", tag="e")
        for sc in range(SC):
            nc.tensor.transpose(
                at_ps[:, sc : sc + 1],
                a_row[0:1, sc * P : (sc + 1) * P],
                ident1,
            )
        atT = atp.tile([P, SC], BF16, name="atT", tag="atT")
        nc.vector.tensor_copy(atT, at_ps)
        return atT

    CG = min(4, SC)  # context col-pack width
    CR = SC // CG

    def context_pass(b, atT, knbs_b):
        pc4s = [
            ps_c.tile([P, chn], F32, name=f"pc4_{i}", tag="c")
            for i, (o, chn) in enumerate(CTX_CH)
        ]
        for rr in range(CR):
            for i, (o, chn) in enumerate(CTX_CH):
                for jj in range(CG):
                    sc = rr * CG + jj
                    j2, nb = divmod(sc, NB)
                    nc.tensor.matmul(
                        pc4s[i][32 * jj : 32 * jj + 1, :],
                        atT[:, sc : sc + 1],
                        knbs_b[j2][:, nb, o : o + chn],
                        start=(rr == 0),
                        stop=(rr == CR - 1),
                        skip_group_check=True,
                        tile_position=(0, 32 * jj),
                    )
        ctx4 = stg.tile([1, K], F32, name="ctx4", tag="crow")
        for i, (o, chn) in enumerate(CTX_CH):
            sl = ctx4[0:1, o : o + chn]
            nc.scalar.copy(sl, pc4s[i][0:1, :])
            for jj in range(1, CG):
                nc.vector.tensor_add(sl, sl, pc4s[i][32 * jj : 32 * jj + 1, :])
        nc.sync.dma_start(out=ctx_out[b : b + 1, :], in_=ctx4[0:1, :])

    # ---------- main schedule ----------
    def new_b_tiles(b):
        e = stg.tile([1, S], F32, name="e_row4", tag="erow")
        m = stg.tile([1, S], U8, name="m_row", tag="mrow")
        nc.sync.dma_start(out=m, in_=mask[b : b + 1, :])
        return e, m

    blocks = [(b, j) for b in range(B) for j in range(JB)]
    LOOKAHEAD = 2
    e_rows, m_rows, a_rows = {}, {}, {}
    knbs, kts = {}, {}
    e_rows[0], m_rows[0] = new_b_tiles(0)
    for i in range(min(LOOKAHEAD, len(blocks))):
        kts[blocks[i]], knbs[blocks[i]] = start_block(*blocks[i])
    carry = None
    for idx, (b, j) in enumerate(blocks):
        if idx + LOOKAHEAD < len(blocks):
            nxt_blk = blocks[idx + LOOKAHEAD]
            kts[nxt_blk], knbs[nxt_blk] = start_block(*nxt_blk)
        carry = compute_block(b, j, kts.pop((b, j)), e_rows[b], carry)
        if j == 0 and b > 0:
            context_pass(
                b - 1,
                attn_transpose(b - 1, a_rows[b - 1]),
                [knbs.pop((b - 1, jj)) for jj in range(JB)],
            )
        if j == JB - 1:
            if carry is not None:
                flush_carry(carry)
                carry = None
            a_rows[b] = softmax(b, e_rows[b], m_rows[b])
            if b + 1 < B:
                e_rows[b + 1], m_rows[b + 1] = new_b_tiles(b + 1)
    context_pass(
        B - 1,
        attn_transpose(B - 1, a_rows[B - 1]),
        [knbs.pop((B - 1, jj)) for jj in range(JB)],
    )


def build_kernel(B=FULL_B // N_CORES, S=FULL_S, K=FULL_K, H=FULL_H, SB=512):
    nc = bacc.Bacc(
        "TRN2", target_bir_lowering=False, debug=False, enable_partition_id=False
    )
    io = {
        "query": nc.dram_tensor("query", [B, K], F32, kind="ExternalInput"),
        "keys": nc.dram_tensor("keys", [B, S, K], F32, kind="ExternalInput"),
        "mask": nc.dram_tensor("mask", [B, S], U8, kind="ExternalInput"),
        "Wq": nc.dram_tensor("Wq", [H, K], F32, kind="ExternalInput"),
        "Wk": nc.dram_tensor("Wk", [H, K], F32, kind="ExternalInput"),
        "v": nc.dram_tensor("v", [1, H], F32, kind="ExternalInput"),
        "context": nc.dram_tensor("context", [B, K], F32, kind="ExternalOutput"),
        "attn": nc.dram_tensor("attn", [B, S], F32, kind="ExternalOutput"),
    }
    with tile.TileContext(nc) as tc:
        with ExitStack() as ctx:
            _emit(ctx, tc, io, B, S, K, H, SB)
    nc.compile()
    return nc


def ref_np(query, keys, mask, Wq, Wk, v):
    """Numpy reference (mirrors the jax oracle) for dev testing."""
    qp = query.astype(np.float64) @ Wq.T.astype(np.float64)
    kp = np.einsum("bsk,hk->bsh", keys, Wk, dtype=np.float64)
    sc = np.tanh(qp[:, None, :] + kp)
    en = np.einsum("bsh,h->bs", sc, v[0].astype(np.float64))
    en = np.where(mask, -np.inf, en)
    mx = np.max(en, axis=-1, keepdims=True)
    mx = np.where(np.isfinite(mx), mx, 0.0)
    ex = np.exp(en - mx)
    sm = ex.sum(axis=-1, keepdims=True)
    attn = np.where(sm > 0, ex / np.where(sm == 0, 1.0, sm), 0.0)
    ctxo = np.einsum("bs,bsk->bk", attn, keys, dtype=np.float64)
    return ctxo.astype(np.float32), attn.astype(np.float32)


_CACHE = {}


def _get_nc():
    if "nc" not in _CACHE:
        _CACHE["nc"] = build_kernel()
    return _CACHE["nc"]


def kernel(query, keys, mask, Wq, Wk, v):
    query = np.ascontiguousarray(np.asarray(query), dtype=np.float32)
    keys = np.ascontiguousarray(np.asarray(keys), dtype=np.float32)
    mask_u8 = np.ascontiguousarray(np.asarray(mask)).astype(np.uint8)
    Wq = np.ascontiguousarray(np.asarray(Wq), dtype=np.float32)
    Wk = np.ascontiguousarray(np.asarray(Wk), dtype=np.float32)
    v = np.ascontiguousarray(np.asarray(v), dtype=np.float32)

    nc = _get_nc()
    bs = FULL_B // N_CORES
    in_maps = []
    for c in range(N_CORES):
        sl = slice(c * bs, (c + 1) * bs)
        in_maps.append(
            {
                "query": np.ascontiguousarray(query[sl]),
                "keys": np.ascontiguousarray(keys[sl]),
                "mask": np.ascontiguousarray(mask_u8[sl]),
                "Wq": Wq,
                "Wk": Wk,
                "v": v,
            }
        )
    res = bass_utils.run_bass_kernel_spmd(nc, in_maps, core_ids=list(range(N_CORES)))
    context = np.concatenate([r["context"] for r in res.results], axis=0)
    attn = np.concatenate([r["attn"] for r in res.results], axis=0)
    return context, attn
